# revision 1
# baseline (speedup 1.0000x reference)
"""GCNConv Trainium2 Bass kernel (8 NeuronCores, SPMD).

out = D_in^-1/2 A D_out^-1/2 X W + b for the deterministic degree-regular
circulant graph (node i -> (i + off_j) % N).  Strategy:
  - shard nodes across 8 cores; replicate W (pre-scaled by 1/DEG on host)
  - phase A: h_loc = Xt_shard.T @ W on each core (fp16 inputs, fp32 psum)
  - AllGather h (fp16) -> h_full, pad h_full[N:N+SH_PAD] = h_full[0:SH_PAD]
  - phase B: out rows = bias + sum_j h_full[start_j : start_j + SH_PAD]
    where start_j = (core*SH + off_j) % N -- the circulant structure turns
    the per-edge gather into 16 contiguous shifted-window reads (dynamic-
    offset DMAs with per-core starts), accumulated on DVE in fp32.
Non-circulant inputs fall back to a host implementation (never taken for
this problem's deterministic inputs).
"""

from contextlib import ExitStack

import numpy as np

import concourse.bacc as bacc
import concourse.bass as bass
import concourse.mybir as mybir
import concourse.tile as tile
from concourse.bass import ds, ts


def build_gcn_nc(
    N: int,          # total nodes
    DIN: int,        # input feature dim
    D: int,          # output feature dim
    DEG: int,        # degree (number of circulant offsets)
    M: int,          # number of cores
    dve_windows: bool = True,
    reps: int = 1,   # body repetitions (for wall-clock delta timing)
    phases: str = "ABW",  # A=matmul, B=allgather+pad, W=windows (subset for bisection)
):
    """Returns (nc, meta). Inputs per core: xt [DIN, SH_PAD] f32 (zero-padded),
    w [DIN, D] f32 (pre-scaled), bias_rep [1, RPB*D] f32, starts [DEG, 1] int32.
    Output: out [SH_PAD, D] f32 (rows >= SH are garbage, host trims)."""
    assert N % M == 0
    SH = N // M                      # shard rows
    RPB = (SH + 127) // 128          # row-blocks per partition
    SH_PAD = 128 * RPB               # padded shard rows
    FREE = RPB * D                   # acc free-dim size
    NT = SH_PAD // 128               # number of 128-row tiles in shard
    KC = (DIN + 127) // 128          # contraction chunks
    assert DIN % KC == 0
    KSZ = DIN // KC                  # contraction chunk size (<=128)
    assert SH_PAD <= N, "window pad must fit one wrap"

    f32 = mybir.dt.float32
    i32 = mybir.dt.int32

    nc = bacc.Bacc("TRN2", num_devices=M)

    f16 = mybir.dt.float16
    xt = nc.dram_tensor("xt", [DIN, SH_PAD], f16, kind="ExternalInput")
    w = nc.dram_tensor("w", [DIN, D], f16, kind="ExternalInput")
    bias_rep = nc.dram_tensor("bias_rep", [1, FREE], f32, kind="ExternalInput")
    starts = nc.dram_tensor("starts", [DEG, 1], i32, kind="ExternalInput")
    out = nc.dram_tensor("out", [SH_PAD, D], f32, kind="ExternalOutput")

    # internal DRAM
    h_loc = nc.dram_tensor("h_loc", [SH * D], f16)
    # flat padded gather buffer; AllGather writes [0 : N*D)
    h_fullp = nc.dram_tensor("h_fullp", [(N + SH_PAD) * D], f16, addr_space="Shared")

    # how many output-column groups fit in one PSUM bank (free dim 512 f32)
    TPG = max(1, min(512 // D, NT))  # tiles per psum group

    with tile.TileContext(nc) as tc, ExitStack() as ctx:
        pconst = ctx.enter_context(tc.tile_pool(name="pconst", bufs=1))
        phg = ctx.enter_context(tc.tile_pool(name="phg", bufs=3))
        psum = ctx.enter_context(tc.tile_pool(name="psum", bufs=1, space="PSUM"))
        pacc = ctx.enter_context(tc.tile_pool(name="pacc", bufs=1))
        pwin = ctx.enter_context(tc.tile_pool(name="pwin", bufs=2))
        for _rep in range(reps):
            # ---------------- phase A: h_loc = (xt.T @ w) ----------------

            # w stored partition-chunked: w_sb[:, c*D:(c+1)*D] = w[c*KSZ:(c+1)*KSZ, :]
            # single DMA via 3D AP so consumers wait on one semaphore
            w_sb = pconst.tile([KSZ, KC * D], f16, tag="w")
            w_r = w.rearrange("(c p) d -> p c d", c=KC, p=KSZ)
            nc.sync.dma_start(out=w_sb[:], in_=w_r[:, :, :])

            # xt fully SBUF-resident in fp16 (one DMA, no slot reuse -> every
            # matmul has <=1 sync wait; PE Matmult/direct-DMA encodings allow 1)
            xt_sb = pconst.tile([KSZ, KC * SH_PAD], f16, tag="xts")
            xt_r = xt.rearrange("(c p) s -> p c s", c=KC, p=KSZ)
            nc.sync.dma_start(out=xt_sb[:], in_=xt_r[:, :, :])

            # ONE psum tile (7 banks = 56 strips) reused for both supergroups
            # (same tile -> no slot-release waits, only region deps):
            #  - sg0 strips 0..b0-1 -> regions 0..b0-1 ascending; tail strips in
            #    the top bank, which sg1 never touches
            #  - sg1 strips -> regions DESCENDING from main-chunk top: the
            #    boundary matmul's single drain wait covers all later ones
            #  - an artificial dep pre-absorbs the main-copy DVE tick
            SGMAX = (4096 - 512) // D  # 7 banks; 1 bank for scr
            BANK = 512 // D            # strips per psum bank
            if NT <= SGMAX:
                bounds = [0, NT]
                MAIN = NT
            else:
                b0 = ((SGMAX - 1) // BANK) * BANK
                b0 = b0 + (SGMAX - b0) // 2
                b0 = min(b0, SGMAX - 2)
                MAIN = (min(b0, SGMAX) // BANK) * BANK
                assert NT - b0 <= MAIN - 2, "sg1 too large for main banks"
                bounds = [0, b0, NT]

            def region_of(t):
                if t < bounds[1]:
                    return t
                return MAIN - 1 - (t - bounds[1])

            scr = psum.tile([128, 8], f32, tag="scr")
            dummy0 = nc.tensor.matmul(
                out=scr[0:1, 0:1], lhsT=w_sb[0:1, 0:1], rhs=w_sb[0:1, 0:1],
                start=True, stop=True,
            )

            chunks = []
            if len(bounds) == 3:
                chunks.append((0, MAIN))       # main: strips==regions, full banks
                chunks.append((MAIN, bounds[1]))  # tail: top-bank strips
                chunks.append((bounds[1], NT))    # sg1 (permuted regions)
            else:
                chunks.append((0, NT))
            copy_after = {hi - 1: (lo, hi) for (lo, hi) in chunks}

            pt = psum.tile([128, SGMAX * D], f32, tag="pt")
            copies, mms = {}, {}
            for t in range(NT):
                r = region_of(t)
                for c in range(KC):
                    mm = nc.tensor.matmul(
                        out=pt[:, r * D:(r + 1) * D],
                        lhsT=xt_sb[:, c * SH_PAD + t * 128: c * SH_PAD + (t + 1) * 128],
                        rhs=w_sb[:, c * D:(c + 1) * D],
                        start=(c == 0),
                        stop=(c == KC - 1),
                    )
                    mms[(t, c)] = mm
                if len(bounds) == 3 and t == bounds[1] - 2:
                    # pre-absorb main-copy DVE tick before the boundary matmul
                    tile.add_dep_helper(
                        mms[(t, 0)].ins, copies[MAIN - 1].ins, sync=True,
                        reason="absorb main-copy DVE tick",
                    )
                if t in copy_after:
                    lo, hi = copy_after[t]
                    regs = sorted(region_of(u) for u in range(lo, hi))
                    rlo, rhi = regs[0], regs[-1] + 1
                    assert regs == list(range(rlo, rhi))
                    hg = phg.tile([128, SGMAX * D], f16, tag="hg", name=f"hg{lo}")
                    cp = nc.vector.tensor_copy(
                        out=hg[:, :(rhi - rlo) * D],
                        in_=pt[:, rlo * D:rhi * D],
                    )
                    copies[t] = cp
                    # batched store: ONE DMA for all full strips of the chunk
                    # (3D AP over h_loc; stride negative for descending regions),
                    # plus one small DMA for a partial last strip if present
                    strips = [u for u in range(lo, hi) if SH - u * 128 > 0]
                    full = [u for u in strips if SH - u * 128 >= 128]
                    partial = [u for u in strips if u not in full]
                    desc = region_of(lo) > region_of(lo + 1) if hi - lo > 1 else False
                    if full:
                        v = [region_of(u) - rlo for u in full]
                        if desc:
                            vmin = min(v)
                            u_at_vmin = full[v.index(vmin)]
                            out_ap = bass.AP(
                                h_loc, u_at_vmin * 128 * D,
                                [[D, 128], [-128 * D, len(full)], [1, D]],
                            )
                        else:
                            out_ap = bass.AP(
                                h_loc, full[0] * 128 * D,
                                [[D, 128], [128 * D, len(full)], [1, D]],
                            )
                            vmin = v[0]
                        nc.sync.dma_start(
                            out=out_ap,
                            in_=hg[:, vmin * D:(vmin + len(full)) * D],
                        )
                    for u in partial:
                        nr = SH - u * 128
                        ro = region_of(u) - rlo
                        nc.sync.dma_start(
                            out=bass.AP(h_loc, u * 128 * D, [[D, nr], [1, D]]),
                            in_=hg[:nr, ro * D:ro * D + D],
                        )

            # ---------------- AllGather + pad ----------------
            if "B" not in phases:
                continue
            nc.gpsimd.collective_compute(
                "AllGather",
                mybir.AluOpType.bypass,
                replica_groups=[list(range(M))],
                ins=[h_loc.ap().opt()],
                outs=[h_fullp[0:N * D].opt()],
            )
            nc.gpsimd.dma_start(out=h_fullp[N * D:(N + SH_PAD) * D], in_=h_fullp[0:SH_PAD * D])

            # ---------------- phase B: shifted-window accumulate ----------------
            if "W" not in phases:
                continue
            acc = pacc.tile([128, FREE // 2], f32, tag="acc")
            # bias tile: DMA-replicate the [1, FREE] row across all 128 partitions
            bt = pacc.tile([128, FREE], f32, tag="bias")
            nc.sync.dma_start(out=bt[:], in_=bias_rep[0:1, :].to_broadcast([128, FREE]))

            # <=6 dynamic-offset DMAs per engine (lowering limit), so spread over 3
            # each window is split into column halves, one per HWDGE
            # engine: doubles DMA queue parallelism and runs two independent
            # DVE accumulation chains (finer pipelining)
            assert FREE % 2 == 0
            FH = FREE // 2
            engs = [nc.sync, nc.scalar]
            accs = [acc, pacc.tile([128, FH], f32, tag="acc1", name="acc1")]
            for j in range(DEG):
                for hh in range(2):
                    eng = engs[hh]
                    wt = pwin.tile([128, FH], f16, tag=f"win{hh}",
                                   name=f"win{j}_{hh}")
                    with eng.register(f"st{_rep}_{j}_{hh}") as reg:
                        eng.reg_load(reg, starts[j:j + 1, 0:1])
                        sv = eng.snap(reg, min_val=0, max_val=N - 1)
                        eng.dma_start(
                            out=wt[:],
                            in_=bass.AP(h_fullp, sv * D + hh * FH,
                                        [[FREE, 128], [1, FH]]),
                        )
                    # j==0 also folds in the bias: acc_h = bt_h + wt0_h
                    a_in = bt[:, hh * FH:(hh + 1) * FH] if j == 0 \
                        else accs[hh][:, :]
                    nc.vector.tensor_add(out=accs[hh][:], in0=a_in, in1=wt[:])

            for hh in range(2):
                nc.sync.dma_start(
                    out=bass.AP(out, hh * FH, [[FREE, 128], [1, FH]]),
                    in_=accs[hh][:],
                )


    nc.compile()
    meta = dict(SH=SH, SH_PAD=SH_PAD, RPB=RPB, FREE=FREE)
    return nc, meta


def make_inputs(N, DIN, D, DEG, M, x, weight, bias, offsets, scale):
    """Host-side input prep. x [N, DIN], weight [DIN, D], bias [D],
    offsets [DEG] int, scale folded into w."""
    SH = N // M
    RPB = (SH + 127) // 128
    SH_PAD = 128 * RPB
    xt_full = np.ascontiguousarray(x.T).astype(np.float16)  # [DIN, N]
    w_eff = (weight.astype(np.float32) * np.float32(scale)).astype(np.float16)
    bias_rep = np.tile(bias.astype(np.float32), RPB)[None, :]
    in_maps = []
    for k in range(M):
        xt_k = np.zeros((DIN, SH_PAD), np.float16)
        xt_k[:, :SH] = xt_full[:, k * SH:(k + 1) * SH]
        starts_k = ((k * SH + offsets) % N).astype(np.int32)[:, None]
        in_maps.append({
            "xt": xt_k,
            "w": w_eff,
            "bias_rep": bias_rep,
            "starts": starts_k,
        })
    return in_maps


# ---------------------------------------------------------------------------
# Host-side entry point
# ---------------------------------------------------------------------------

_CACHE = {}


def _get_nc(N, DIN, D, DEG, M):
    key = (N, DIN, D, DEG, M)
    if key not in _CACHE:
        _CACHE[key] = build_gcn_nc(N, DIN, D, DEG, M)
    return _CACHE[key]


def _is_circulant(N, DEG, rowptr, colind, colptr):
    if rowptr.shape[0] != N + 1 or colind.shape[0] != N * DEG:
        return None
    if not np.array_equal(rowptr.astype(np.int64),
                          np.arange(N + 1, dtype=np.int64) * DEG):
        return None
    if not np.array_equal(colptr, rowptr):
        return None
    offsets = colind[:DEG].astype(np.int64)
    if offsets.min() < 1 or offsets.max() >= N or len(set(offsets.tolist())) != DEG:
        return None
    rows = np.arange(N, dtype=np.int64)
    expect = np.sort((rows[:, None] + offsets[None, :]) % N, axis=1).reshape(-1)
    if not np.array_equal(colind.astype(np.int64), expect):
        return None
    return offsets


def _kernel_numpy_fallback(x, weight, bias, rowptr, colind, colptr):
    # general-graph fallback (never taken for the deterministic circulant
    # inputs this problem generates; correctness insurance only)
    h = x.astype(np.float32) @ weight.astype(np.float32)
    out_deg = (colptr[1:] - colptr[:-1]).astype(np.float32)
    in_deg = (rowptr[1:] - rowptr[:-1]).astype(np.float32)
    h = h * (1.0 / np.sqrt(np.maximum(out_deg, 1e-30)))[:, None]
    N = rowptr.shape[0] - 1
    E = colind.shape[0]
    row_ids = np.searchsorted(rowptr, np.arange(E), side="right") - 1
    aggr = np.zeros_like(h)
    np.add.at(aggr, row_ids, h[colind])
    aggr = aggr * (1.0 / np.sqrt(np.maximum(in_deg, 1e-30)))[:, None]
    return (aggr + bias).astype(np.float32)


def kernel(x, weight, bias, rowptr, colind, colptr, rowind=None, **_unused):
    """GCNConv: out = D_in^-1/2 A D_out^-1/2 X W + b, distributed over 8
    NeuronCores (nodes sharded; h AllGathered; circulant-shift windows)."""
    from concourse.bass_utils import run_bass_kernel_spmd

    x = np.asarray(x)
    weight = np.asarray(weight)
    bias = np.asarray(bias)
    rowptr = np.asarray(rowptr)
    colind = np.asarray(colind)
    colptr = np.asarray(colptr)

    N, DIN = x.shape
    D = weight.shape[1]
    M = 8
    DEG = colind.shape[0] // max(N, 1)

    offsets = _is_circulant(N, DEG, rowptr, colind, colptr)
    if offsets is None or N % M != 0 or DIN % 128 != 0:
        return _kernel_numpy_fallback(x, weight, bias, rowptr, colind, colptr)

    # degree-regular graph: both rsqrt scalings are exactly 1/DEG (powers of
    # two for DEG=16), folded into W on the host
    scale = 1.0 / DEG

    nc, meta = _get_nc(N, DIN, D, DEG, M)
    in_maps = make_inputs(N, DIN, D, DEG, M, x, weight, bias, offsets, scale)
    res = run_bass_kernel_spmd(nc, in_maps, list(range(M)))
    SH = meta["SH"]
    out = np.concatenate(
        [np.asarray(res.results[k]["out"])[:SH] for k in range(M)], axis=0
    )
    return out.astype(np.float32)



# revision 4
# speedup vs baseline: 23.0352x; 23.0352x over previous
"""GCNConv Trainium2 Bass kernel (8 NeuronCores, SPMD).

out = D_in^-1/2 A D_out^-1/2 X W + b for the deterministic degree-regular
circulant graph (node i -> (i + off_j) % N).  Strategy:
  - shard nodes across 8 cores; replicate W (pre-scaled by 1/DEG on host)
  - phase A: h_loc = Xt_shard.T @ W on each core (fp16 inputs, fp32 psum)
  - AllGather h (fp16) -> h_full, pad h_full[N:N+SH_PAD] = h_full[0:SH_PAD]
  - phase W: out rows = bias + sum_j h_full[start_j : start_j + SH_PAD]
    where start_j = (core*SH + off_j) % N -- the circulant structure turns
    the per-edge gather into 16 contiguous shifted-window reads (dynamic-
    offset DMAs with per-core starts).  The 16-window accumulation is
    split across engines: PE accumulates the first column half into PSUM
    via identity matmuls (layout-agnostic, fp32), DVE accumulates the
    second half in fp16.  Outputs are two DRAM tensors (fp32 + fp16)
    reassembled on the host.
Non-circulant inputs fall back to a host implementation (never taken for
this problem's deterministic inputs).
"""

from contextlib import ExitStack

import numpy as np

import concourse.bacc as bacc
import concourse.bass as bass
import concourse.mybir as mybir
import concourse.tile as tile
from concourse.bass import ds, ts


def build_gcn_nc(
    N: int,          # total nodes
    DIN: int,        # input feature dim
    D: int,          # output feature dim
    DEG: int,        # degree (number of circulant offsets)
    M: int,          # number of cores
    reps: int = 1,   # body repetitions (for wall-clock delta timing)
    phases: str = "ABW",  # A=matmul, B=allgather+pad, W=windows (subset for bisection)
):
    """Returns (nc, meta). Inputs per core: xt [DIN, SH_PAD] f16 (zero-padded),
    w [DIN, D] f16 (pre-scaled), biasp16 [1, FREE] f16, eye128 [128,128] f16,
    ones1 [1,128] f16, starts [DEG, 1] int32.
    Outputs: out_pe [128*FH] f32 (rows p*98+[0,49)), out_ve [128*FH] f16
    (rows p*98+[49,98)); host trims rows >= SH."""
    assert N % M == 0
    SH = N // M                      # shard rows
    RPB = (SH + 127) // 128          # row-blocks per partition
    SH_PAD = 128 * RPB               # padded shard rows
    FREE = RPB * D                   # acc free-dim size
    NT = SH_PAD // 128               # number of 128-row tiles in shard
    KC = (DIN + 127) // 128          # contraction chunks
    assert DIN % KC == 0
    KSZ = DIN // KC                  # contraction chunk size (<=128)
    assert SH_PAD <= N, "window pad must fit one wrap"

    f32 = mybir.dt.float32
    i32 = mybir.dt.int32

    nc = bacc.Bacc("TRN2", num_devices=M)

    f16 = mybir.dt.float16
    xt = nc.dram_tensor("xt", [DIN, SH_PAD], f16, kind="ExternalInput")
    w = nc.dram_tensor("w", [DIN, D], f16, kind="ExternalInput")
    biasp16 = nc.dram_tensor("biasp16", [1, FREE], f16, kind="ExternalInput")
    eye128 = nc.dram_tensor("eye128", [128, 128], f16, kind="ExternalInput")
    ones1 = nc.dram_tensor("ones1", [1, 128], f16, kind="ExternalInput")
    starts = nc.dram_tensor("starts", [DEG, 1], i32, kind="ExternalInput")
    assert FREE % 2 == 0
    FH = FREE // 2
    out_pe = nc.dram_tensor("out_pe", [128 * FH], f32, kind="ExternalOutput")
    out_ve = nc.dram_tensor("out_ve", [128 * FH], f16, kind="ExternalOutput")

    # internal DRAM
    h_loc = nc.dram_tensor("h_loc", [SH * D], f16)
    # flat padded gather buffer; AllGather writes [0 : N*D)
    h_fullp = nc.dram_tensor("h_fullp", [(N + SH_PAD) * D], f16, addr_space="Shared")

    with tile.TileContext(nc) as tc, ExitStack() as ctx:
        pconst = ctx.enter_context(tc.tile_pool(name="pconst", bufs=1))
        phg = ctx.enter_context(tc.tile_pool(name="phg", bufs=3))
        psum = ctx.enter_context(tc.tile_pool(name="psum", bufs=1, space="PSUM"))
        pacc = ctx.enter_context(tc.tile_pool(name="pacc", bufs=1))
        pwin = ctx.enter_context(tc.tile_pool(name="pwin", bufs=3))
        for _rep in range(reps):
            # ---------------- phase A: h_loc = (xt.T @ w) ----------------

            # w stored partition-chunked: w_sb[:, c*D:(c+1)*D] = w[c*KSZ:(c+1)*KSZ, :]
            # single DMA via 3D AP so consumers wait on one semaphore
            w_sb = pconst.tile([KSZ, KC * D], f16, tag="w")
            w_r = w.rearrange("(c p) d -> p c d", c=KC, p=KSZ)
            nc.sync.dma_start(out=w_sb[:], in_=w_r[:, :, :])

            # xt fully SBUF-resident in fp16 (one DMA, no slot reuse -> every
            # matmul has <=1 sync wait; PE Matmult/direct-DMA encodings allow 1)
            xt_sb = pconst.tile([KSZ, KC * SH_PAD], f16, tag="xts")
            xt_r = xt.rearrange("(c p) s -> p c s", c=KC, p=KSZ)
            nc.sync.dma_start(out=xt_sb[:], in_=xt_r[:, :, :])

            # constants for phase W (loaded early, overlap with phase A)
            eye_sb = pconst.tile([128, 128], f16, tag="eye")
            nc.scalar.dma_start(out=eye_sb[:], in_=eye128[:, :])
            ones_sb = pconst.tile([1, 128], f16, tag="ones")
            nc.scalar.dma_start(out=ones_sb[:], in_=ones1[:, :])
            biasp_sb = pconst.tile([1, FREE], f16, tag="biasp")
            nc.scalar.dma_start(out=biasp_sb[:], in_=biasp16[:, :])

            # ONE psum tile (7 banks = 56 strips) reused for both supergroups
            # (same tile -> no slot-release waits, only region deps):
            #  - sg0 strips 0..b0-1 -> regions 0..b0-1 ascending; tail strips in
            #    the top bank, which sg1 never touches
            #  - sg1 strips -> regions DESCENDING from main-chunk top: the
            #    boundary matmul's single drain wait covers all later ones
            #  - an artificial dep pre-absorbs the main-copy DVE tick
            SGMAX = (4096 - 512) // D  # 7 banks; 1 bank for scr
            BANK = 512 // D            # strips per psum bank
            if NT <= SGMAX:
                bounds = [0, NT]
                MAIN = NT
            else:
                b0 = ((SGMAX - 1) // BANK) * BANK
                b0 = b0 + (SGMAX - b0) // 2
                b0 = min(b0, SGMAX - 2)
                MAIN = (min(b0, SGMAX) // BANK) * BANK
                assert NT - b0 <= MAIN - 2, "sg1 too large for main banks"
                bounds = [0, b0, NT]

            def region_of(t):
                if t < bounds[1]:
                    return t
                return MAIN - 1 - (t - bounds[1])

            scr = psum.tile([128, 8], f32, tag="scr")
            dummy0 = nc.tensor.matmul(
                out=scr[0:1, 0:1], lhsT=w_sb[0:1, 0:1], rhs=w_sb[0:1, 0:1],
                start=True, stop=True,
            )

            chunks = []
            if len(bounds) == 3:
                chunks.append((0, MAIN))       # main: strips==regions, full banks
                chunks.append((MAIN, bounds[1]))  # tail: top-bank strips
                chunks.append((bounds[1], NT))    # sg1 (permuted regions)
            else:
                chunks.append((0, NT))
            copy_after = {hi - 1: (lo, hi) for (lo, hi) in chunks}

            pt = psum.tile([128, SGMAX * D], f32, tag="pt")
            copies, mms = {}, {}
            for t in range(NT):
                r = region_of(t)
                for c in range(KC):
                    mm = nc.tensor.matmul(
                        out=pt[:, r * D:(r + 1) * D],
                        lhsT=xt_sb[:, c * SH_PAD + t * 128: c * SH_PAD + (t + 1) * 128],
                        rhs=w_sb[:, c * D:(c + 1) * D],
                        start=(c == 0),
                        stop=(c == KC - 1),
                    )
                    mms[(t, c)] = mm
                if len(bounds) == 3 and t == bounds[1] - 2:
                    # pre-absorb main-copy DVE tick before the boundary matmul
                    tile.add_dep_helper(
                        mms[(t, 0)].ins, copies[MAIN - 1].ins, sync=True,
                        reason="absorb main-copy DVE tick",
                    )
                if t in copy_after:
                    lo, hi = copy_after[t]
                    regs = sorted(region_of(u) for u in range(lo, hi))
                    rlo, rhi = regs[0], regs[-1] + 1
                    assert regs == list(range(rlo, rhi))
                    hg = phg.tile([128, SGMAX * D], f16, tag="hg", name=f"hg{lo}")
                    cp = nc.vector.tensor_copy(
                        out=hg[:, :(rhi - rlo) * D],
                        in_=pt[:, rlo * D:rhi * D],
                    )
                    copies[t] = cp
                    # batched store: ONE DMA for all full strips of the chunk
                    # (3D AP over h_loc; stride negative for descending regions),
                    # plus one small DMA for a partial last strip if present
                    strips = [u for u in range(lo, hi) if SH - u * 128 > 0]
                    full = [u for u in strips if SH - u * 128 >= 128]
                    partial = [u for u in strips if u not in full]
                    desc = region_of(lo) > region_of(lo + 1) if hi - lo > 1 else False
                    if full:
                        v = [region_of(u) - rlo for u in full]
                        if desc:
                            vmin = min(v)
                            u_at_vmin = full[v.index(vmin)]
                            out_ap = bass.AP(
                                h_loc, u_at_vmin * 128 * D,
                                [[D, 128], [-128 * D, len(full)], [1, D]],
                            )
                        else:
                            out_ap = bass.AP(
                                h_loc, full[0] * 128 * D,
                                [[D, 128], [128 * D, len(full)], [1, D]],
                            )
                            vmin = v[0]
                        nc.sync.dma_start(
                            out=out_ap,
                            in_=hg[:, vmin * D:(vmin + len(full)) * D],
                        )
                    for u in partial:
                        nr = SH - u * 128
                        ro = region_of(u) - rlo
                        nc.sync.dma_start(
                            out=bass.AP(h_loc, u * 128 * D, [[D, nr], [1, D]]),
                            in_=hg[:nr, ro * D:ro * D + D],
                        )

            # ---------------- AllGather + pad ----------------
            if "B" not in phases:
                continue
            nc.gpsimd.collective_compute(
                "AllGather",
                mybir.AluOpType.bypass,
                replica_groups=[list(range(M))],
                ins=[h_loc.ap().opt()],
                outs=[h_fullp[0:N * D].opt()],
            )
            nc.gpsimd.dma_start(out=h_fullp[N * D:(N + SH_PAD) * D], in_=h_fullp[0:SH_PAD * D])

            # ---------------- phase W: shifted-window accumulate ----------------
            # half 0 (free [0, FH)): PE identity-matmul accumulation into PSUM
            # half 1 (free [FH, FREE)): DVE fp16 adds
            if "W" not in phases:
                continue

            # bias tile for DVE half: broadcast biasp16[0, FH:FREE] to 128 parts
            bt = pacc.tile([128, FH], f16, tag="bias")
            nc.scalar.dma_start(
                out=bt[:], in_=biasp16[0:1, FH:FREE].to_broadcast([128, FH]))

            # PE psum accumulator (shares the "pt" slot with phase A: the
            # first matmul below waits for phase A's last psum drain)
            ptW = psum.tile([128, SGMAX * D], f32, tag="pt", name="ptW")
            # bias seed: ptW[p, f] = ones[0, p] * biasp[0, f]
            CH = 512  # psum-bank-aligned matmul chunks
            wchunks = [(c, min(c + CH, FH)) for c in range(0, FH, CH)]
            for (c0, c1) in wchunks:
                nc.tensor.matmul(
                    out=ptW[:, c0:c1], lhsT=ones_sb[:, :],
                    rhs=biasp_sb[0:1, c0:c1], start=True, stop=False,
                )

            acc = pacc.tile([128, FH], f16, tag="acc")

            engs = [nc.sync, nc.scalar]
            for j in range(DEG):
                # --- gather both halves of window j (dynamic starts) ---
                wts = []
                for hh in range(2):
                    eng = engs[hh]
                    wt = pwin.tile([128, FH], f16, tag=f"win{hh}",
                                   name=f"win{j}_{hh}")
                    with eng.register(f"st{_rep}_{j}_{hh}") as reg:
                        eng.reg_load(reg, starts[j:j + 1, 0:1])
                        sv = eng.snap(reg, min_val=0, max_val=N - 1)
                        eng.dma_start(
                            out=wt[:],
                            in_=bass.AP(h_fullp, sv * D + hh * FH,
                                        [[FREE, 128], [1, FH]]),
                        )
                    wts.append(wt)
                # --- PE: accumulate half 0 into psum ---
                for (c0, c1) in wchunks:
                    nc.tensor.matmul(
                        out=ptW[:, c0:c1], lhsT=eye_sb[:, :],
                        rhs=wts[0][:, c0:c1], start=False, stop=(j == DEG - 1),
                    )
                # --- DVE: accumulate half 1 in fp16 ---
                a_in = bt[:, :] if j == 0 else acc[:, :]
                nc.vector.tensor_add(out=acc[:], in0=a_in, in1=wts[1][:])

            # stores (PSUM cannot DMA directly; drain via Activation copy)
            pe_out = pacc.tile([128, FH], f32, tag="peout")
            nc.scalar.copy(out=pe_out[:], in_=ptW[:, 0:FH])
            nc.sync.dma_start(
                out=bass.AP(out_pe, 0, [[FH, 128], [1, FH]]),
                in_=pe_out[:],
            )
            nc.scalar.dma_start(
                out=bass.AP(out_ve, 0, [[FH, 128], [1, FH]]),
                in_=acc[:],
            )


    nc.compile()
    meta = dict(SH=SH, SH_PAD=SH_PAD, RPB=RPB, FREE=FREE, FH=FH)
    return nc, meta


def make_inputs(N, DIN, D, DEG, M, x, weight, bias, offsets, scale):
    """Host-side input prep. x [N, DIN], weight [DIN, D], bias [D],
    offsets [DEG] int, scale folded into w."""
    SH = N // M
    RPB = (SH + 127) // 128
    SH_PAD = 128 * RPB
    FREE = RPB * D
    xt_full = np.ascontiguousarray(x.T).astype(np.float16)  # [DIN, N]
    w_eff = (weight.astype(np.float32) * np.float32(scale)).astype(np.float16)
    biasp16 = np.tile(bias.astype(np.float16), RPB)[None, :]
    eye128 = np.eye(128, dtype=np.float16)
    ones1 = np.ones((1, 128), dtype=np.float16)
    in_maps = []
    for k in range(M):
        xt_k = np.zeros((DIN, SH_PAD), np.float16)
        xt_k[:, :SH] = xt_full[:, k * SH:(k + 1) * SH]
        starts_k = ((k * SH + offsets) % N).astype(np.int32)[:, None]
        in_maps.append({
            "xt": xt_k,
            "w": w_eff,
            "biasp16": biasp16,
            "eye128": eye128,
            "ones1": ones1,
            "starts": starts_k,
        })
    return in_maps


def assemble_output(res, M, SH, RPB, D):
    """Reassemble full output from per-core out_pe (f32) / out_ve (f16)."""
    HB = RPB // 2  # 49 row-blocks per half
    parts = []
    for k in range(M):
        ope = np.asarray(res.results[k]["out_pe"]).reshape(128, HB, D)
        ove = np.asarray(res.results[k]["out_ve"]).astype(np.float32)
        ove = ove.reshape(128, HB, D)
        full = np.concatenate([ope, ove], axis=1).reshape(128 * RPB * D)
        parts.append(full.reshape(-1, D)[:SH])
    return np.concatenate(parts, axis=0)


# ---------------------------------------------------------------------------
# Host-side entry point
# ---------------------------------------------------------------------------

_CACHE = {}


def _get_nc(N, DIN, D, DEG, M):
    key = (N, DIN, D, DEG, M)
    if key not in _CACHE:
        _CACHE[key] = build_gcn_nc(N, DIN, D, DEG, M)
    return _CACHE[key]


def _is_circulant(N, DEG, rowptr, colind, colptr):
    if rowptr.shape[0] != N + 1 or colind.shape[0] != N * DEG:
        return None
    if not np.array_equal(rowptr.astype(np.int64),
                          np.arange(N + 1, dtype=np.int64) * DEG):
        return None
    if not np.array_equal(colptr, rowptr):
        return None
    offsets = colind[:DEG].astype(np.int64)
    if offsets.min() < 1 or offsets.max() >= N or len(set(offsets.tolist())) != DEG:
        return None
    rows = np.arange(N, dtype=np.int64)
    expect = np.sort((rows[:, None] + offsets[None, :]) % N, axis=1).reshape(-1)
    if not np.array_equal(colind.astype(np.int64), expect):
        return None
    return offsets


def _kernel_numpy_fallback(x, weight, bias, rowptr, colind, colptr):
    # general-graph fallback (never taken for the deterministic circulant
    # inputs this problem generates; correctness insurance only)
    h = x.astype(np.float32) @ weight.astype(np.float32)
    out_deg = (colptr[1:] - colptr[:-1]).astype(np.float32)
    in_deg = (rowptr[1:] - rowptr[:-1]).astype(np.float32)
    h = h * (1.0 / np.sqrt(np.maximum(out_deg, 1e-30)))[:, None]
    N = rowptr.shape[0] - 1
    E = colind.shape[0]
    row_ids = np.searchsorted(rowptr, np.arange(E), side="right") - 1
    aggr = np.zeros_like(h)
    np.add.at(aggr, row_ids, h[colind])
    aggr = aggr * (1.0 / np.sqrt(np.maximum(in_deg, 1e-30)))[:, None]
    return (aggr + bias).astype(np.float32)


def kernel(x, weight, bias, rowptr, colind, colptr, rowind=None, **_unused):
    """GCNConv: out = D_in^-1/2 A D_out^-1/2 X W + b, distributed over 8
    NeuronCores (nodes sharded; h AllGathered; circulant-shift windows)."""
    from concourse.bass_utils import run_bass_kernel_spmd

    x = np.asarray(x)
    weight = np.asarray(weight)
    bias = np.asarray(bias)
    rowptr = np.asarray(rowptr)
    colind = np.asarray(colind)
    colptr = np.asarray(colptr)

    N, DIN = x.shape
    D = weight.shape[1]
    M = 8
    DEG = colind.shape[0] // max(N, 1)

    offsets = _is_circulant(N, DEG, rowptr, colind, colptr)
    if offsets is None or N % M != 0 or DIN % 128 != 0:
        return _kernel_numpy_fallback(x, weight, bias, rowptr, colind, colptr)

    # degree-regular graph: both rsqrt scalings are exactly 1/DEG (powers of
    # two for DEG=16), folded into W on the host
    scale = 1.0 / DEG

    nc, meta = _get_nc(N, DIN, D, DEG, M)
    in_maps = make_inputs(N, DIN, D, DEG, M, x, weight, bias, offsets, scale)
    res = run_bass_kernel_spmd(nc, in_maps, list(range(M)))
    out = assemble_output(res, M, meta["SH"], meta["RPB"], D)
    return out.astype(np.float32)
